# revision 36
# baseline (speedup 1.0000x reference)
"""Trainium2 Bass kernel: multi-head attention block (B=4, N=2048, C=1024, H=16).

Sharding: 8 cores = (batch b in 0..3) x (head-group hg in 0..1, 8 heads each).
Each core computes qkv for its heads, full attention for its heads over its
batch, and a partial projection (its 512 rows of W_proj). Host sums the two
partials per batch and adds b_proj.

Device layout choices (all matmuls bf16 inputs, fp32 PSUM accumulate):
  - q,k produced in transposed layout qkT[dim, token] so S^T = k^T-chunks.T @ q^T
    needs no on-chip transposes.
  - v produced in natural layout [token, 65*h] with a trailing ones column per
    head, so the O matmul lhsT=[v|ones] gives row 64 = softmax denominator and
    rows 0..63 = unnormalized o^T in one PSUM accumulation chain.
  - exp on ScalarE with fused scale=1/8; no max subtraction (logits bounded).
  - head PAIRS: the two heads' S matmuls sit at base partitions 0/64 (distinct
    PE row groups) and issue back-to-back, so they run concurrently.
  - attention is ACT(exp)-bound; qk-projection and output-projection matmuls
    are interleaved into the j-loops between the S and O matmuls, so they
    issue while the O matmuls head-of-line wait on the exp semaphore.
"""

import os
import sys
from contextlib import ExitStack

import numpy as np
import ml_dtypes

import concourse.bass as bass
import concourse.tile as tile
from concourse import bacc, mybir
from concourse.bass import ds, ts
from concourse.bass_utils import run_bass_kernel_spmd

try:  # without the NTFF hook module, a stray BASS_TRACE=1 would crash the run
    from antenv.axon_hooks import get_axon_ntff_profile_hook  # noqa: F401
except ImportError:
    os.environ.setdefault("BASS_NEVER_TRACE", "1")

BF16 = mybir.dt.bfloat16
F32 = mybir.dt.float32
NP_BF16 = ml_dtypes.bfloat16

B, N, C = 4, 2048, 1024
H, D = 16, 64
HPC = 8            # heads per core
CD = HPC * D       # 512 local qkv dims per core
E = D + 1          # 65: 64 v dims + ones column

LAST_RESULTS = None  # stash for test harness (exec_time_ns, trace paths)


def _build_program(taps=False):
    nc = bacc.Bacc("TRN2", target_bir_lowering=False, debug=False)

    xT_d = nc.dram_tensor("xT", [C, N], BF16, kind="ExternalInput").ap()
    wqk_d = nc.dram_tensor("wqk", [C, 2 * CD], BF16, kind="ExternalInput").ap()
    wv_d = nc.dram_tensor("wv", [C, CD], BF16, kind="ExternalInput").ap()
    bqk_d = nc.dram_tensor("bqk", [128, 8], F32, kind="ExternalInput").ap()
    bv_d = nc.dram_tensor("bv", [1, CD], BF16, kind="ExternalInput").ap()
    wp_d = nc.dram_tensor("wp", [CD, C], BF16, kind="ExternalInput").ap()
    out_d = nc.dram_tensor("out", [N, C], F32, kind="ExternalOutput").ap()
    if taps:
        tap_qkT = nc.dram_tensor("tap_qkT", [128, 8, N], BF16, kind="ExternalOutput").ap()
        tap_v = nc.dram_tensor("tap_v", [128, 16, HPC * E], BF16, kind="ExternalOutput").ap()
        tap_ex = nc.dram_tensor("tap_ex", [128, 1024], BF16, kind="ExternalOutput").ap()
        tap_ot = nc.dram_tensor("tap_ot", [65, 512], F32, kind="ExternalOutput").ap()
        tap_recb = nc.dram_tensor("tap_recb", [64, 512], F32, kind="ExternalOutput").ap()
        tap_oT = nc.dram_tensor("tap_oT", [128, 4, N], BF16, kind="ExternalOutput").ap()

    with tile.TileContext(nc) as tc, ExitStack() as ctx:
        singles = ctx.enter_context(tc.tile_pool(name="singles", bufs=1))
        ps_pool = ctx.enter_context(tc.tile_pool(name="ps", bufs=2, space="PSUM"))
        st_pool = ctx.enter_context(tc.tile_pool(name="stp", bufs=2, space="PSUM"))
        ot_pool = ctx.enter_context(tc.tile_pool(name="ot", bufs=2, space="PSUM"))
        exp_pool = ctx.enter_context(tc.tile_pool(name="expp", bufs=4))
        misc = ctx.enter_context(tc.tile_pool(name="misc", bufs=4))
        ob_pool = ctx.enter_context(tc.tile_pool(name="ob", bufs=3))

        # Persistent SBUF tensors, chunk-major: [partition, chunk, free].
        xT_sb = singles.tile([128, 8, N], BF16)        # x^T   [c, token]
        wqk_sb = singles.tile([128, 8, 2 * CD], BF16)  # W_qk  [c, m]
        wv_sb = singles.tile([128, 8, CD], BF16)       # W_v   [c, n]
        bqk_sb = singles.tile([128, 8], F32)
        bv_sb = singles.tile([1, CD], BF16)
        ones_sb = singles.tile([1, 128], BF16)
        wp_sb = singles.tile([128, 4, C], BF16)        # W_proj [hd, n]
        qkT_sb = singles.tile([128, 8, N], BF16)       # chunks 0..3 = q, 4..7 = k
        v_sb = singles.tile([128, 16, HPC * E], BF16)  # [token-in-chunk, tchunk, h*(64+1)]
        oT_sb = singles.tile([128, 4, N], BF16)        # o^T, proj lhsT layout

        for kc in range(8):
            nc.sync.dma_start(wv_sb[:, kc], wv_d.rearrange("(c p) m -> p c m", p=128)[:, kc])
        nc.sync.dma_start(bv_sb, bv_d)
        for kc in range(8):
            nc.sync.dma_start(xT_sb[:, kc], xT_d.rearrange("(c p) t -> p c t", p=128)[:, kc])
        nc.sync.dma_start(wqk_sb, wqk_d.rearrange("(c p) m -> p c m", p=128))
        nc.sync.dma_start(bqk_sb, bqk_d)
        nc.sync.dma_start(wp_sb, wp_d.rearrange("(c p) n -> p c n", p=128))
        nc.vector.memset(ones_sb, 1.0)

        # v natural layout + bias via ones x bv matmul.
        def emit_v_chunk(t):
            ps = st_pool.tile([128, 512], F32, tag="st")
            for kc in range(8):
                nc.tensor.matmul(
                    ps,
                    xT_sb[:, kc, ts(t, 128)],
                    wv_sb[:, kc, :],
                    start=(kc == 0),
                    stop=False,
                )
            nc.tensor.matmul(ps, ones_sb, bv_sb, start=False, stop=True)
            vv = v_sb[:, t].rearrange("p (h e) -> p h e", e=E)
            nc.vector.tensor_copy(vv[:, :, 0:D], ps.rearrange("p (h d) -> p h d", d=D))
            nc.vector.memset(vv[:, :, D : D + 1], 1.0)

        for t in range(16):
            emit_v_chunk(t)

        # qk chunk m as a list of per-matmul thunks (so they can be spread
        # through an earlier pair's j-loop as PE filler).
        def qk_chunk_thunks(m):
            thunks = []
            for i4q in range(4):
                hold = {}

                def mk(kc, m=m, i4q=i4q, hold=hold):
                    def f():
                        if kc == 0:
                            hold["ps"] = ps_pool.tile([128, 512], F32, tag="big", name="qkps")
                        nc.tensor.matmul(
                            hold["ps"],
                            wqk_sb[:, kc, ts(m, 128)],
                            xT_sb[:, kc, ds(i4q * 512, 512)],
                            start=(kc == 0),
                            stop=(kc == 7),
                        )
                        if kc == 7:
                            nc.vector.tensor_scalar_add(
                                qkT_sb[:, m, ds(i4q * 512, 512)],
                                hold["ps"],
                                bqk_sb[:, ds(m, 1)],
                            )

                    return f

                for kc in range(8):
                    thunks.append(mk(kc))
            return thunks

        def emit_proj(t):
            for nh in range(2):
                pp = ps_pool.tile([128, 512], F32, tag="big")
                for hc in range(4):
                    nc.tensor.matmul(
                        pp,
                        oT_sb[:, hc, ts(t, 128)],
                        wp_sb[:, hc, ds(nh * 512, 512)],
                        start=(hc == 0),
                        stop=(hc == 3),
                    )
                ob = ob_pool.tile([128, 512], F32)
                nc.vector.tensor_copy(ob, pp)
                nc.sync.dma_start(out_d[ts(t, 128), ds(nh * 512, 512)], ob)


        # Phase 2: per head PAIR. The two heads' S matmuls target different PE
        # row groups (base partitions 0 / 64) so back-to-back issue runs them
        # concurrently; both write one [128, 1024] st tile and share one exp.
        for p in range(4):
            qA, qB = qkT_sb[0:64, p], qkT_sb[64:128, p]
            kA, kB = qkT_sb[0:64, 4 + p], qkT_sb[64:128, 4 + p]
            hA, hB = 2 * p, 2 * p + 1
            # PE filler emitted between S and O each step: next pair's qk
            # matmuls (pairs 0..2), or earlier quarters' projections (pair 3).
            for th in qk_chunk_thunks(p):
                th()
            for th in qk_chunk_thunks(4 + p):
                th()
            for i4 in range(4):
                i0 = i4 * 512
                otA = ot_pool.tile([65, 512], F32, tag="ot")
                otB = ot_pool.tile([65, 512], F32, tag="ot")
                for j in range(16):
                    st = st_pool.tile([128, 1024], F32, tag="st")
                    nc.tensor.matmul(
                        st[:, 0:512], kA[:, ts(j, 128)], qA[:, ds(i0, 512)],
                        start=True, stop=True,
                    )
                    nc.tensor.matmul(
                        st[:, 512:1024], kB[:, ts(j, 128)], qB[:, ds(i0, 512)],
                        start=True, stop=True,
                    )
                    ex = exp_pool.tile([128, 1024], BF16)
                    nc.scalar.activation(
                        ex, st, mybir.ActivationFunctionType.Exp, scale=float(D) ** -0.5
                    )
                    if taps and p == 0 and i4 == 0 and j == 0:
                        nc.sync.dma_start(tap_ex, ex)
                    vvj = v_sb[:, j].rearrange("p (h e) -> p h e", e=E)
                    nc.tensor.matmul(
                        otA, vvj[:, hA], ex[:, 0:512], start=(j == 0), stop=(j == 15)
                    )
                    nc.tensor.matmul(
                        otB, vvj[:, hB], ex[:, 512:1024], start=(j == 0), stop=(j == 15)
                    )
                for hp_, ot in ((0, otA), (64, otB)):
                    # Copy the whole accumulator out first: frees the PSUM slot
                    # fast; same DVE cost as one row (partitions are parallel).
                    otc = misc.tile([65, 512], F32, tag="otc")
                    nc.vector.tensor_copy(otc, ot)
                    if taps and p == 0 and i4 == 0 and hp_ == 0:
                        nc.sync.dma_start(tap_ot, otc)
                    # Softmax denominators: lane-scatter so reciprocal runs on
                    # 128 lanes x 4 elems instead of 1 lane x 512 (DVE divide
                    # is ~8 cycles/elem serial per lane).
                    s_t = misc.tile([128, 4], F32, tag="sct")
                    nc.sync.dma_start(s_t, otc[64:65])
                    r_t = misc.tile([128, 4], F32, tag="rct")
                    nc.vector.reciprocal(r_t, s_t)
                    rec0 = misc.tile([1, 512], F32, tag="rec0")
                    nc.sync.dma_start(rec0, r_t)
                    recb = misc.tile([64, 512], F32, tag="recb")
                    nc.gpsimd.partition_broadcast(recb, rec0)
                    if taps and p == 0 and i4 == 0 and hp_ == 0:
                        nc.sync.dma_start(tap_recb, recb)
                    tmp = misc.tile([64, 512], BF16, tag="tmp")
                    nc.vector.tensor_mul(tmp, otc[0:64], recb)
                    nc.sync.dma_start(oT_sb[hp_ : hp_ + 64, p, ds(i0, 512)], tmp)

        if taps:
            nc.sync.dma_start(tap_qkT, qkT_sb)
            nc.sync.dma_start(tap_v, v_sb)
            nc.sync.dma_start(tap_oT, oT_sb)

        for t in range(16):
            emit_proj(t)

    nc.compile()
    return nc


_PROGRAM = None


def kernel(x, W_qkv, b_qkv, W_proj, b_proj):
    global _PROGRAM, LAST_RESULTS
    x = np.asarray(x, dtype=np.float32)
    W_qkv = np.asarray(W_qkv, dtype=np.float32)
    b_qkv = np.asarray(b_qkv, dtype=np.float32)
    W_proj = np.asarray(W_proj, dtype=np.float32)
    b_proj = np.asarray(b_proj, dtype=np.float32)

    if _PROGRAM is None:
        _PROGRAM = _build_program()
    nc = _PROGRAM

    in_maps = []
    for core in range(8):
        b, hg = core // 2, core % 2
        h0 = hg * HPC
        sl = slice(h0 * D, h0 * D + CD)
        wq = W_qkv[:, 0 * C :][:, sl]
        wk = W_qkv[:, 1 * C :][:, sl]
        wv = W_qkv[:, 2 * C :][:, sl]
        bq = b_qkv[0 * C :][sl]
        bk = b_qkv[1 * C :][sl]
        bv = b_qkv[2 * C :][sl]
        in_maps.append(
            {
                "xT": np.ascontiguousarray(x[b].T).astype(NP_BF16),
                "wqk": np.concatenate([wq, wk], axis=1).astype(NP_BF16),
                "wv": np.ascontiguousarray(wv).astype(NP_BF16),
                "bqk": np.concatenate([bq, bk]).reshape(8, 128).T.astype(np.float32).copy(),
                "bv": bv.reshape(1, CD).astype(NP_BF16),
                "wp": np.ascontiguousarray(W_proj[sl, :]).astype(NP_BF16),
            }
        )

    res = run_bass_kernel_spmd(nc, in_maps, list(range(8)))
    LAST_RESULTS = res
    out = np.empty((B, N, C), dtype=np.float32)
    for b in range(B):
        out[b] = (
            res.results[2 * b]["out"].astype(np.float32)
            + res.results[2 * b + 1]["out"].astype(np.float32)
            + b_proj[None, :]
        )
    return out
